# revision 8
# baseline (speedup 1.0000x reference)
"""DLPT layer (2 blocks of per-cluster LPE-MLPs + local self-attention) on 8 trn2 cores.

Sharding: data-parallel over batch B=4 (2 cores per batch element), each core
takes a contiguous half of the N=16384 points (clusters are contiguous index
ranges per the arange cluster_idx fill).  Each core runs both DLPT blocks on
its 8192 points; the final FPS gather runs on host.

Device layout strategy:
  - activations token-major [128 tok, d] for LayerNorm stats (bn_stats) and
    normalize (tensor_scalar with per-partition mean/rstd)
  - matmul inputs feature-major [d, tok] (lhsT / rhs), produced by PE
    transposes whose PSUM eviction fuses the LN affine + ReLU on the scalar
    engine (per-partition scale/bias in feature-major)
  - attention computed transposed: L^T = K·Q^T so softmax denominators land
    per-partition and fold into the PV eviction; Wv@Wo folded on host
"""

import sys

sys.path.insert(0, "/opt/trn_rl_repo")

import numpy as np
import ml_dtypes

import concourse.bass as bass
import concourse.tile as tile
from concourse import bacc, mybir, bass_utils
from concourse.bass import ts
from concourse.masks import make_identity

BF16 = mybir.dt.bfloat16
F32 = mybir.dt.float32
AF = mybir.ActivationFunctionType
ALU = mybir.AluOpType

B, N = 4, 16384
P_PTS = 8192            # points per core
NT = P_PTS // 128       # 64 token tiles per core
ST_TOK = 1024           # super-tile size (tokens)
NST = P_PTS // ST_TOK   # 8 super-tiles
EPS = 1e-5

# (F, D, S): fc dim, embed dim, cluster size
CFG1 = dict(F=64, D=256, S=64)
CFG2 = dict(F=256, D=512, S=256)


# ----------------------------------------------------------------------------
# device module
# ----------------------------------------------------------------------------

def _ln_stats(nc, pool, psum_y, eps_t):
    """bn_stats path on a [128, d] tile -> (neg_mean*rstd bias, rstd) [128,1]."""
    st = pool.tile([128, 6], F32, tag="ln_st")
    nc.vector.bn_stats(out=st, in_=psum_y)
    mv = pool.tile([128, 2], F32, tag="ln_mv")
    nc.vector.bn_aggr(out=mv, in_=st)
    sd = pool.tile([128, 1], F32, tag="ln_sd")
    # sd = sqrt(var + eps)
    nc.scalar.activation(out=sd, in_=mv[:, 1:2], func=AF.Sqrt, bias=eps_t, scale=1.0)
    rs = pool.tile([128, 1], F32, tag="ln_rs")
    nc.vector.reciprocal(out=rs, in_=sd)
    return mv[:, 0:1], rs


def _block(nc, tc, ctx, cfg, io):
    """Emit one DLPT block. io: dict with APs (SBUF tiles / DRAM aps)."""
    F, D, S = cfg["F"], cfg["D"], cfg["S"]
    KD = D // 128                  # feature chunks of D
    KF = F // 128 if F >= 128 else None

    xa = io["xa"]                  # [5, 8192] bf16 sbuf (local3, ones, nrm)
    fcT = io["fcT"]                # [128, F/128, 8192] or [64, 8192] bf16 sbuf
    ident = io["ident"]            # [128,128] bf16 identity
    ones_row = io["ones_row"]      # [1, 8192] bf16
    ones_col = io["ones_col"]      # [128, 1] bf16
    eps_t = io["eps_t"]            # [128, 1] f32 = eps
    w = io["w"]                    # dict of weight sbuf tiles
    out_dram = io["out_dram"]      # [8192, D] dram ap (bf16 for b1 / f32 for b2)
    tag = io["tag"]

    NC_ST = ST_TOK // S            # clusters per super-tile
    TS_CH = S // 128 if S >= 128 else None  # 128-chunks per cluster (t dim)

    small = ctx.enter_context(tc.tile_pool(name=f"{tag}_small", bufs=4))
    zpool = ctx.enter_context(tc.tile_pool(name=f"{tag}_z", bufs=3))
    psum = io["p_big"]
    psum_t = io["p_tr"]
    psum_a = io["p_tr"]
    psum_one = io["p_one"]

    # persistent per-super-tile feature-major tensors
    stp = ctx.enter_context(tc.tile_pool(name=f"{tag}_stp", bufs=1))

    for st in range(NST):
        t0 = st * ST_TOK
        rT = stp.tile([64, ST_TOK], BF16, tag="rT")
        rhT = stp.tile([64, ST_TOK], BF16, tag="rhT")
        h_posT = stp.tile([128, KD, ST_TOK], BF16, tag="h_posT")
        h_geoT = stp.tile([128, KD, ST_TOK], BF16, tag="h_geoT")
        h_pos_tok = stp.tile([128, ST_TOK // 128, D], BF16, tag="h_pos_tok")
        qT = stp.tile([128, KD, ST_TOK], BF16, tag="qT")
        kT = stp.tile([128, KD, ST_TOK], BF16, tag="kT")
        vp = stp.tile([128, ST_TOK // 128, D], BF16, tag="vp")

        # ---- MLP-a (4->64): r and r_hat ----
        for i in range(ST_TOK // 128):
            tsl = slice(t0 + i * 128, t0 + (i + 1) * 128)
            for which, wa, ga, bea, dstT in (
                ("r", w["wa1"], w["ga1"], w["bea1"], rT),
                ("rh", w["wa2"], w["ga2"], w["bea2"], rhT),
            ):
                krows = 5 if which == "r" else 4
                py = psum.tile([128, 512], F32, tag="big", name="pya")[:, 0:64]
                nc.tensor.matmul(py, xa[0:krows, tsl], wa[0:krows, :],
                                 start=True, stop=True)
                m, rs = _ln_stats(nc, small, py, eps_t)
                z = zpool.tile([128, 64], BF16, tag="za")
                nc.vector.tensor_scalar(out=z, in0=py, scalar1=m, scalar2=rs,
                                        op0=ALU.subtract, op1=ALU.mult)
                pt = psum_t.tile([128, 128], BF16, tag="tr")
                nc.tensor.transpose(pt[0:64, :], z, ident)
                nc.scalar.activation(out=dstT[:, i * 128:(i + 1) * 128],
                                     in_=pt[0:64, :], func=AF.Relu,
                                     bias=bea, scale=ga)

        # ---- MLP-b x2: h_pos (j=0), h_geo (j=1) ----
        for j, (wbr, wbf, bb, gcol, becol, gb, beb, hT) in enumerate((
            (w["wbr1"], w["wbf1"], w["bb1"], w["g1col"], w["be1col"],
             w.get("g1_b"), w.get("be1_b"), h_posT),
            (w["wbr2"], w["wbf2"], w["bb2"], w["g2col"], w["be2col"],
             None, None, h_geoT),
        )):
            rsrc = rT if j == 0 else rhT
            for i in range(ST_TOK // 128):
                isl = slice(i * 128, (i + 1) * 128)
                tsl = slice(t0 + i * 128, t0 + (i + 1) * 128)
                py = psum.tile([128, 512], F32, tag="big", name="pbig")[:, 0:D]
                nc.tensor.matmul(py, rsrc[:, isl], wbr, start=True, stop=False)
                if KF is None:
                    nc.tensor.matmul(py, fcT[:, tsl], wbf[0:F, :],
                                     start=False, stop=False)
                else:
                    for fk in range(KF):
                        nc.tensor.matmul(py, fcT[:, fk, tsl],
                                         wbf[:, fk, :], start=False, stop=False)
                nc.tensor.matmul(py, ones_row[:, tsl], bb,
                                 start=False, stop=True)
                m, rs = _ln_stats(nc, small, py, eps_t)
                z = zpool.tile([128, D], BF16, tag="zb")
                nc.vector.tensor_scalar(out=z, in0=py, scalar1=m, scalar2=rs,
                                        op0=ALU.subtract, op1=ALU.mult)
                # feature-major copy with fused affine+relu on transpose-evict
                for kc in range(KD):
                    pt = psum_t.tile([128, 128], BF16, tag="tr")
                    nc.tensor.transpose(pt, z[:, kc * 128:(kc + 1) * 128], ident)
                    nc.scalar.activation(
                        out=hT[:, kc, i * 128:(i + 1) * 128], in_=pt,
                        func=AF.Relu, bias=becol[:, kc:kc + 1],
                        scale=gcol[:, kc:kc + 1])
                if j == 0:
                    # token-major h_pos for the residual: affine+relu in 3 passes
                    t1 = zpool.tile([128, D], BF16, tag="t1")
                    nc.vector.tensor_tensor(out=t1, in0=z, in1=gb, op=ALU.mult)
                    t2 = zpool.tile([128, D], BF16, tag="t2")
                    nc.vector.tensor_tensor(out=t2, in0=t1, in1=beb, op=ALU.add)
                    nc.vector.tensor_scalar_max(h_pos_tok[:, i, :], t2, 0.0)

        # ---- Q^T, K^T (form B, weight-stationary) ----
        for wqk, dT in ((w["wq"], qT), (w["wk"], kT)):
            for mc in range(KD):
                for nh in range(ST_TOK // 512):
                    nsl = slice(nh * 512, (nh + 1) * 512)
                    pq = psum.tile([128, 512], F32, tag="big")
                    for kc in range(KD):
                        nc.tensor.matmul(
                            pq, wqk[:, kc, mc * 128:(mc + 1) * 128],
                            h_geoT[:, kc, nsl], start=(kc == 0),
                            stop=(kc == KD - 1))
                    nc.scalar.copy(out=dT[:, mc, nsl], in_=pq)

        # ---- V' = h_pos @ (Wv Wo) (form A) ----
        for i in range(ST_TOK // 128):
            isl = slice(i * 128, (i + 1) * 128)
            pv = psum.tile([128, 512], F32, tag="big", name="pbig")[:, 0:D]
            for kc in range(KD):
                nc.tensor.matmul(pv, h_posT[:, kc, isl], w["wvo"][:, kc, :],
                                 start=(kc == 0), stop=(kc == KD - 1))
            nc.scalar.copy(out=vp[:, i, :], in_=pv)

        # ---- attention + residual + ln1 + store ----
        # S=64: two clusters per 128-tile, block-diagonal E^T; S=256: 2x2 chunks
        def ln1_tail(at_psum, rd, ti, trow):
            at = zpool.tile([128, D], BF16, tag="at")
            nc.vector.tensor_scalar_mul(at, at_psum, rd)
            res = zpool.tile([128, D], BF16, tag="res")
            nc.vector.tensor_tensor(out=res, in0=at, in1=w["bo_b"], op=ALU.add)
            nc.vector.tensor_tensor(out=res, in0=res, in1=h_pos_tok[:, ti, :],
                                    op=ALU.add)
            m, rs2 = _ln_stats(nc, small, res, eps_t)
            zf = zpool.tile([128, D], BF16, tag="zf")
            nc.vector.tensor_scalar(out=zf, in0=res, scalar1=m, scalar2=rs2,
                                    op0=ALU.subtract, op1=ALU.mult)
            o1 = zpool.tile([128, D], BF16, tag="o1")
            nc.vector.tensor_tensor(out=o1, in0=zf, in1=w["gl_b"], op=ALU.mult)
            of = zpool.tile([128, D], out_dram.dtype, tag="of")
            nc.vector.tensor_tensor(out=of, in0=o1, in1=w["bel_b"], op=ALU.add)
            nc.sync.dma_start(out=out_dram[trow:trow + 128, :], in_=of)

        if TS_CH is None:
            # S=64: process cluster pairs (one 128-token tile each)
            for ti in range(ST_TOK // 128):
                p0 = ti * 128
                eT = zpool.tile([128, 128], BF16, tag="eT")
                nc.gpsimd.memset(eT, 0.0)
                for half in range(2):
                    hsl = slice(p0 + half * 64, p0 + half * 64 + 64)
                    osl = slice(half * 64, half * 64 + 64)
                    pl = psum.tile([128, 512], F32, tag="big", name="pl64")
                    for kc in range(KD):
                        nc.tensor.matmul(pl[osl, osl], kT[:, kc, hsl],
                                         qT[:, kc, hsl], start=(kc == 0),
                                         stop=(kc == KD - 1),
                                         tile_position=(0, half * 64))
                    nc.scalar.activation(out=eT[osl, osl], in_=pl[osl, osl],
                                         func=AF.Exp)
                pd = psum_one.tile([128, 1], F32, tag="one")
                nc.tensor.matmul(pd, eT, ones_col, start=True, stop=True)
                rd = small.tile([128, 1], F32, tag="rd")
                nc.vector.reciprocal(out=rd, in_=pd)
                pat = psum.tile([128, 512], F32, tag="big", name="pat")[:, 0:D]
                nc.tensor.matmul(pat, eT, vp[:, ti, :], start=True, stop=True)
                ln1_tail(pat, rd, ti, t0 + p0)
        else:
            # S=256: 2 t-chunks x 2 s-chunks per cluster
            for c in range(NC_ST):
                c0 = c * S
                eT = zpool.tile([128, TS_CH, S], BF16, tag="eT2")
                for tch in range(TS_CH):
                    pl = psum.tile([128, 512], F32, tag="big", name="plS")[:, 0:S]
                    ksl = slice(c0 + tch * 128, c0 + (tch + 1) * 128)
                    for kc in range(KD):
                        nc.tensor.matmul(pl, kT[:, kc, ksl],
                                         qT[:, kc, c0:c0 + S],
                                         start=(kc == 0), stop=(kc == KD - 1))
                    nc.scalar.activation(out=eT[:, tch, :], in_=pl, func=AF.Exp)
                for sc in range(TS_CH):
                    ssl = slice(sc * 128, (sc + 1) * 128)
                    pd = psum_one.tile([128, 1], F32, tag="one")
                    for tch in range(TS_CH):
                        nc.tensor.matmul(pd, eT[:, tch, ssl], ones_col,
                                         start=(tch == 0), stop=(tch == TS_CH - 1))
                    rd = small.tile([128, 1], F32, tag="rd2")
                    nc.vector.reciprocal(out=rd, in_=pd)
                    pat = psum.tile([128, 512], F32, tag="big", name="pat2")[:, 0:D]
                    for tch in range(TS_CH):
                        ti = (c0 + tch * 128) // 128
                        nc.tensor.matmul(pat, eT[:, tch, ssl], vp[:, ti, :],
                                         start=(tch == 0), stop=(tch == TS_CH - 1))
                    ln1_tail(pat, rd, (c0 + sc * 128) // 128, t0 + c0 + sc * 128)


def _load_weights(nc, tc, ctx, cfg, tag, dram):
    """DMA block weights into SBUF; returns dict of tiles."""
    F, D = cfg["F"], cfg["D"]
    KD = D // 128
    KF = F // 128 if F >= 128 else None
    wp = ctx.enter_context(tc.tile_pool(name=f"w_{tag}", bufs=1))
    w = {}
    def loadt(name, shape, dt, in_ap):
        t = wp.tile(shape, dt, tag=name)
        nc.sync.dma_start(out=t, in_=in_ap)
        w[name] = t
        return t

    loadt("wa1", [5, 64], BF16, dram[f"wa1_{tag}"])
    loadt("wa2", [4, 64], BF16, dram[f"wa2_{tag}"])
    for nm in ("ga1", "bea1", "ga2", "bea2"):
        loadt(nm, [64, 1], F32, dram[f"{nm}_{tag}"].rearrange("(a b) -> a b", b=1))
    loadt("wbr1", [64, D], BF16, dram[f"wbr1_{tag}"])
    loadt("wbr2", [64, D], BF16, dram[f"wbr2_{tag}"])
    if KF is None:
        loadt("wbf1", [F, D], BF16, dram[f"wbf1_{tag}"])
        loadt("wbf2", [F, D], BF16, dram[f"wbf2_{tag}"])
    else:
        loadt("wbf1", [128, KF, D], BF16,
              dram[f"wbf1_{tag}"].rearrange("(c p) d -> p c d", p=128))
        loadt("wbf2", [128, KF, D], BF16,
              dram[f"wbf2_{tag}"].rearrange("(c p) d -> p c d", p=128))
    loadt("bb1", [1, D], BF16, dram[f"bb1_{tag}"].rearrange("(b a) -> b a", b=1))
    loadt("bb2", [1, D], BF16, dram[f"bb2_{tag}"].rearrange("(b a) -> b a", b=1))
    # column layouts [128, KD] for feature-major affine (per-partition scalars)
    for nm in ("g1", "be1", "g2", "be2"):
        loadt(f"{nm}col", [128, KD], F32,
              dram[f"{nm}_{tag}"].rearrange("(c p) -> p c", p=128))
    # broadcast rows [128, D] for token-major affine
    for nm, src in (("g1_b", "g1"), ("be1_b", "be1"),
                    ("gl_b", "gl"), ("bel_b", "bel"), ("bo_b", "bo")):
        dd = dram[f"{src}_{tag}"]
        t = wp.tile([128, D], F32, tag=nm)
        nc.sync.dma_start(out=t, in_=bass.AP(
            tensor=dd.tensor, offset=dd.offset, ap=[[0, 128]] + list(dd.ap)))
        w[nm] = t
    for nm in ("wq", "wk", "wvo"):
        loadt(nm, [128, KD, D], BF16,
              dram[f"{nm}_{tag}"].rearrange("(c p) d -> p c d", p=128))
    return w


def build_module():
    nc = bacc.Bacc("TRN2", target_bir_lowering=False, debug=False, num_devices=8)
    dram = {}

    def din(name, shape, dt):
        dram[name] = nc.dram_tensor(name, shape, dt, kind="ExternalInput").ap()

    for tag, cfg in (("b1", CFG1), ("b2", CFG2)):
        F, D = cfg["F"], cfg["D"]
        din(f"xa_{tag}", [5, P_PTS], BF16)
        din(f"wa1_{tag}", [5, 64], BF16)
        din(f"wa2_{tag}", [4, 64], BF16)
        for nm in ("ga1", "bea1", "ga2", "bea2"):
            din(f"{nm}_{tag}", [64], F32)
        din(f"wbr1_{tag}", [64, D], BF16)
        din(f"wbf1_{tag}", [F, D], BF16)
        din(f"bb1_{tag}", [D], BF16)
        din(f"wbr2_{tag}", [64, D], BF16)
        din(f"wbf2_{tag}", [F, D], BF16)
        din(f"bb2_{tag}", [D], BF16)
        for nm in ("g1", "be1", "g2", "be2", "gl", "bel", "bo"):
            din(f"{nm}_{tag}", [D], F32)
        for nm in ("wq", "wk", "wvo"):
            din(f"{nm}_{tag}", [D, D], BF16)
    din("fcT_b1", [64, P_PTS], BF16)
    out2 = nc.dram_tensor("out_b2", [P_PTS, CFG2["D"]], F32,
                          kind="ExternalOutput").ap()

    import contextlib
    with tile.TileContext(nc) as tc, contextlib.ExitStack() as ctx:
        gp = ctx.enter_context(tc.tile_pool(name="globals", bufs=1))
        dramp = ctx.enter_context(tc.tile_pool(name="dscratch", bufs=1,
                                               space="DRAM"))
        p_big = ctx.enter_context(tc.tile_pool(name="p_big", bufs=3,
                                               space="PSUM"))
        p_tr = ctx.enter_context(tc.tile_pool(name="p_tr", bufs=2,
                                              space="PSUM"))
        p_one = ctx.enter_context(tc.tile_pool(name="p_one", bufs=1,
                                               space="PSUM"))
        ident = gp.tile([128, 128], BF16, tag="ident")
        make_identity(nc, ident)
        ones_row = gp.tile([1, P_PTS], BF16, tag="ones_row")
        nc.vector.memset(ones_row, 1.0)
        ones_col = gp.tile([128, 1], BF16, tag="ones_col")
        nc.vector.memset(ones_col, 1.0)
        eps_t = gp.tile([128, 1], F32, tag="eps_t")
        nc.vector.memset(eps_t, EPS)

        feat1 = dramp.tile([P_PTS, CFG1["D"]], BF16)
        shared = dict(ident=ident, ones_row=ones_row, ones_col=ones_col, eps_t=eps_t,
                      p_big=p_big, p_tr=p_tr, p_one=p_one)

        # block 1
        with contextlib.ExitStack() as bctx:
            w1 = _load_weights(nc, tc, bctx, CFG1, "b1", dram)
            bp = bctx.enter_context(tc.tile_pool(name="b1_in", bufs=1))
            xa1 = bp.tile([5, P_PTS], BF16, tag="xa1")
            nc.sync.dma_start(out=xa1, in_=dram["xa_b1"])
            fcT1 = bp.tile([64, P_PTS], BF16, tag="fcT1")
            nc.sync.dma_start(out=fcT1, in_=dram["fcT_b1"])
            _block(nc, tc, bctx, CFG1, dict(
                xa=xa1, fcT=fcT1, w=w1, out_dram=feat1[:], tag="b1", **shared))

        # block 2 (fc = feat1^T via xbar transpose loads)
        with contextlib.ExitStack() as bctx:
            w2 = _load_weights(nc, tc, bctx, CFG2, "b2", dram)
            bp = bctx.enter_context(tc.tile_pool(name="b2_in", bufs=1))
            xa2 = bp.tile([5, P_PTS], BF16, tag="xa2")
            nc.sync.dma_start(out=xa2, in_=dram["xa_b2"])
            fcT2 = bp.tile([128, CFG2["F"] // 128, P_PTS], BF16, tag="fcT2")
            for c in range(CFG2["F"] // 128):
                nc.sync.dma_start_transpose(fcT2[:, c, :],
                                            feat1[:, c * 128:(c + 1) * 128])
            _block(nc, tc, bctx, CFG2, dict(
                xa=xa2, fcT=fcT2, w=w2, out_dram=out2, tag="b2", **shared))

    nc.compile()
    return nc


# ----------------------------------------------------------------------------
# host side
# ----------------------------------------------------------------------------

def _bf(x):
    return np.asarray(x, np.float32).astype(ml_dtypes.bfloat16)


def _prep_weights(p, cfg, tag):
    """p: dict of np arrays for one block. Returns name->np array (shared)."""
    F, D = cfg["F"], cfg["D"]
    g = lambda k: np.asarray(p[k], np.float32)
    out = {}
    w1a, w2a = g("1a_W"), g("2a_W")
    out[f"wa1_{tag}"] = _bf(np.vstack([w1a[0:3], g("1a_b")[None], w1a[3:4]]))
    out[f"wa2_{tag}"] = _bf(np.vstack([w2a[3:6], g("2a_b")[None]]))
    out[f"ga1_{tag}"] = g("1a_g"); out[f"bea1_{tag}"] = g("1a_be")
    out[f"ga2_{tag}"] = g("2a_g"); out[f"bea2_{tag}"] = g("2a_be")
    w1b, w2b = g("1b_W"), g("2b_W")
    out[f"wbr1_{tag}"] = _bf(w1b[0:64]); out[f"wbf1_{tag}"] = _bf(w1b[64:])
    out[f"bb1_{tag}"] = _bf(g("1b_b"))
    out[f"wbr2_{tag}"] = _bf(w2b[0:64]); out[f"wbf2_{tag}"] = _bf(w2b[64:])
    out[f"bb2_{tag}"] = _bf(g("2b_b"))
    out[f"g1_{tag}"] = g("1b_g"); out[f"be1_{tag}"] = g("1b_be")
    out[f"g2_{tag}"] = g("2b_g"); out[f"be2_{tag}"] = g("2b_be")
    out[f"gl_{tag}"] = g("ln1_g"); out[f"bel_{tag}"] = g("ln1_b")
    out[f"bo_{tag}"] = g("bo")
    out[f"wq_{tag}"] = _bf(g("Wq") / np.sqrt(D))
    out[f"wk_{tag}"] = _bf(g("Wk"))
    out[f"wvo_{tag}"] = _bf(g("Wv") @ g("Wo"))
    return out


def _geom(pos_s, S):
    """pos_s [8192,3] f32 -> xa [5, 8192] bf16 rows [local xyz, ones, nrm]."""
    pc = pos_s.reshape(-1, S, 3)
    cog = pc.mean(axis=1, keepdims=True)
    local = (pc - cog).reshape(-1, 3)
    nrm = np.linalg.norm(local, axis=-1)
    xa = np.empty((5, pos_s.shape[0]), np.float32)
    xa[0:3] = local.T
    xa[3] = 1.0
    xa[4] = nrm
    return _bf(xa)


_NC_CACHE = {}
_last_in_maps = None


def kernel(pos, feat, cluster_idx1, cluster_idx2, fps_idx, params1, params2):
    pos = np.asarray(pos, np.float32)
    feat = np.asarray(feat, np.float32)
    ci1 = np.asarray(cluster_idx1).astype(np.int64)
    ci2 = np.asarray(cluster_idx2).astype(np.int64)
    fps = np.asarray(fps_idx).astype(np.int64)

    ar = np.arange(N, dtype=np.int64)
    assert np.array_equal(ci1.reshape(B, -1), np.broadcast_to(ar, (B, N))), \
        "kernel assumes arange cluster_idx1"
    assert np.array_equal(ci2.reshape(B, -1), np.broadcast_to(ar, (B, N))), \
        "kernel assumes arange cluster_idx2"

    if "nc" not in _NC_CACHE:
        _NC_CACHE["nc"] = build_module()
    nc = _NC_CACHE["nc"]

    shared = {}
    shared.update(_prep_weights(params1, CFG1, "b1"))
    shared.update(_prep_weights(params2, CFG2, "b2"))

    in_maps = []
    for core in range(8):
        b, h = core // 2, core % 2
        sl = slice(h * P_PTS, (h + 1) * P_PTS)
        m = dict(shared)
        m["xa_b1"] = _geom(pos[b, sl], CFG1["S"])
        m["xa_b2"] = _geom(pos[b, sl], CFG2["S"])
        m["fcT_b1"] = _bf(feat[b, sl].T)
        in_maps.append(m)

    global _last_in_maps
    _last_in_maps = in_maps
    res = bass_utils.run_bass_kernel_spmd(nc, in_maps, core_ids=list(range(8)))

    feat2 = np.empty((B, N, CFG2["D"]), np.float32)
    for core in range(8):
        b, h = core // 2, core % 2
        feat2[b, h * P_PTS:(h + 1) * P_PTS] = res.results[core]["out_b2"]

    bidx = np.arange(B)[:, None]
    pos_ds = pos[bidx, fps]
    feat_ds = feat2[bidx, fps]
    return (pos_ds.astype(np.float32), feat_ds.astype(np.float32))


if __name__ == "__main__":
    build_module()
    print("module built ok")
